# revision 3
# baseline (speedup 1.0000x reference)
"""GraphConv (scatter-mean message passing + linear + relu) on 8 trn2 cores.

Strategy (hardcoded for N=100000 nodes, D=128 feats, E=3.2M edges, 8 cores):
  - Host: sort edges by dst; shard dst nodes contiguously (12500/core).
    Per core, group edges by (128-node block, src-range bucket); 4 buckets
    of 25000 rows so row indices fit dma_gather's int16 idxs. Pad every
    (block, bucket) cell to a uniform chunk count C so one SPMD program
    serves all cores.
  - Device, per 128-node block b:
      G  = dma_gather(X[src]) rows for all chunks        [128e, C*4, 128f]
      Sel= is_equal(dst_local, iota)  one-hot             [128e, C*4*128]
      aggT (psum) = sum_j G_j^T @ Sel_j                   [128f, 128n]
      own path: X_own -> PE transpose -> X_ownT
      out = relu(X_own@W_top + b + inv_deg*(agg@W_bot))   [128n, 128fo]
  - deg is a host-side bincount folded in as inv_deg (per-partition scalar).
"""
import numpy as np

import concourse.bacc as bacc
import concourse.bass as bass
import concourse.mybir as mybir
import concourse.tile as tile
from concourse.bass_utils import run_bass_kernel_spmd
from concourse.masks import make_identity

N_NODES = 100000
D = 128
P = 128
NCORES = 8
NPC = N_NODES // NCORES          # 12500 nodes per core
NBLK = (NPC + P - 1) // P        # 98 blocks (last has 84 valid rows)
LAST_VALID = NPC - (NBLK - 1) * P  # 84
NBUCKET = 4
BUCKET = N_NODES // NBUCKET      # 25000 (< int16 max)
BGRP = 2                         # blocks per gather group
NGRP = NBLK // BGRP              # 49

FP = mybir.dt.float32
I32 = mybir.dt.int32
I16 = mybir.dt.int16


def _preprocess(X, src, dst, W, b):
    """Bucket/pad edges; build per-core device arrays. Returns (C, in_maps)."""
    src = np.ascontiguousarray(src.astype(np.int32))
    dst = np.ascontiguousarray(dst.astype(np.int32))
    X = np.ascontiguousarray(X.astype(np.float32))
    W = np.ascontiguousarray(W.astype(np.float32))
    b = np.ascontiguousarray(b.astype(np.float32)).reshape(1, D)

    deg = np.bincount(dst, minlength=N_NODES)
    inv_deg = (1.0 / np.maximum(deg, 1)).astype(np.float32)

    order = np.argsort(dst)
    src_s = src[order]
    dst_s = dst[order]
    core_bounds = np.searchsorted(dst_s, np.arange(NCORES + 1) * NPC)

    # first pass: global max chunks per (core, block, bucket) cell
    cells = []
    for c in range(NCORES):
        lo, hi = core_bounds[c], core_bounds[c + 1]
        s, d = src_s[lo:hi], dst_s[lo:hi] - c * NPC
        key = (d >> 7) * NBUCKET + np.minimum(s // BUCKET, NBUCKET - 1)
        o2 = np.argsort(key, kind="stable")
        s, d, key = s[o2], d[o2], key[o2]
        counts = np.bincount(key, minlength=NBLK * NBUCKET)
        cells.append((s, d, key, counts))
    C = max(1, int(max((cnt.max() for (_, _, _, cnt) in cells)) + 127) // 128)
    CB = C * NBUCKET            # chunks per block
    NIG = BGRP * C * P          # idxs per (group, bucket) gather

    in_maps = []
    for c in range(NCORES):
        s, d, key, counts = cells[c]
        starts = np.zeros(NBLK * NBUCKET, np.int64)
        np.cumsum(counts[:-1], out=starts[1:])
        rank = np.arange(len(s)) - np.repeat(starts, counts)
        slot = np.repeat(np.arange(NBLK * NBUCKET) * (C * P), counts) + rank

        # flat padded layout [NBLK, NBUCKET, C*P]
        idx_flat = np.zeros(NBLK * NBUCKET * C * P, np.int16)
        idx_flat[slot] = (s - (np.minimum(s // BUCKET, NBUCKET - 1) * BUCKET)
                          ).astype(np.int16)
        dlo_flat = np.full(NBLK * NBUCKET * C * P, P, np.int32)
        dlo_flat[slot] = d & 127

        # gather idxs: per (group, bucket) the list L = concat over the
        # BGRP blocks of that cell's padded idx list; position i of L maps
        # to (partition i%128, chunk i//128). SBUF layout is 16-wrapped
        # ([i%16, i//16]) replicated 8x down the 128 partitions.
        iv = idx_flat.reshape(NGRP, BGRP, NBUCKET, C * P)
        L = np.transpose(iv, (0, 2, 1, 3)).reshape(NGRP, NBUCKET, NIG)
        w16 = np.transpose(L.reshape(NGRP, NBUCKET, NIG // 16, 16),
                           (0, 1, 3, 2))              # [NGRP, NBUCKET, 16, NIG//16]
        rep = np.tile(w16, (1, 1, 8, 1))              # [NGRP, NBUCKET, 128, NIG//16]
        idx_arr = np.ascontiguousarray(
            np.transpose(rep, (0, 2, 1, 3)).reshape(NGRP * 128,
                                                    NBUCKET * (NIG // 16)))

        # dst-local one-hot ids: [128, NBLK*CB]; col b*CB + (k*C + j),
        # partition p = edge (chunk j of bucket k in block b, lane p)
        dv = dlo_flat.reshape(NBLK, CB, P)             # [b, k*C+j, p]
        dlo_arr = np.ascontiguousarray(
            np.transpose(dv, (2, 0, 1)).reshape(P, NBLK * CB))

        ivd = inv_deg[c * NPC:(c + 1) * NPC]
        ivd = np.pad(ivd, (0, NBLK * P - NPC), constant_values=1.0)
        ivd_arr = np.ascontiguousarray(ivd.reshape(NBLK, P).T)

        in_maps.append({
            "Xg": X,
            "Xo": np.ascontiguousarray(X[c * NPC:(c + 1) * NPC]),
            "idx": idx_arr,
            "dlo": dlo_arr,
            "ivd": ivd_arr,
            "Wm": W,
            "br": b,
        })
    return C, in_maps


def _build(C):
    CB = C * NBUCKET
    NIG = BGRP * C * P
    nc = bacc.Bacc("TRN2", target_bir_lowering=False, debug=False,
                   num_devices=NCORES)
    Xg = nc.dram_tensor("Xg", [N_NODES, D], FP, kind="ExternalInput").ap()
    Xo = nc.dram_tensor("Xo", [NPC, D], FP, kind="ExternalInput").ap()
    idx_d = nc.dram_tensor("idx", [NGRP * 128, NBUCKET * (NIG // 16)], I16,
                           kind="ExternalInput").ap()
    dlo_d = nc.dram_tensor("dlo", [P, NBLK * CB], I32,
                           kind="ExternalInput").ap()
    ivd_d = nc.dram_tensor("ivd", [P, NBLK], FP, kind="ExternalInput").ap()
    W_d = nc.dram_tensor("Wm", [2 * D, D], FP, kind="ExternalInput").ap()
    br_d = nc.dram_tensor("br", [1, D], FP, kind="ExternalInput").ap()
    out_d = nc.dram_tensor("out", [NPC, D], FP, kind="ExternalOutput").ap()

    with tile.TileContext(nc) as tc:
        with tc.tile_pool(name="const", bufs=1) as cp, \
             tc.tile_pool(name="gpool", bufs=2) as gp, \
             tc.tile_pool(name="ipool", bufs=2) as ip, \
             tc.tile_pool(name="bpool", bufs=3) as bp, \
             tc.tile_pool(name="spool", bufs=2) as sp, \
             tc.tile_pool(name="psum", bufs=2, space="PSUM") as pp:
            ident = cp.tile([P, P], FP)
            make_identity(nc, ident[:])
            iota_t = cp.tile([P, P], I32)
            nc.gpsimd.iota(iota_t[:], pattern=[[1, P]], base=0,
                           channel_multiplier=0)
            ones_t = cp.tile([1, P], FP)
            nc.vector.memset(ones_t[:], 1.0)
            Wt = cp.tile([P, D], FP)
            nc.sync.dma_start(out=Wt[:], in_=W_d[0:D, :])
            Wb = cp.tile([P, D], FP)
            nc.sync.dma_start(out=Wb[:], in_=W_d[D:2 * D, :])
            br_t = cp.tile([1, P], FP)
            nc.sync.dma_start(out=br_t[:], in_=br_d[:])
            ivd_t = cp.tile([P, NBLK], FP)
            nc.sync.dma_start(out=ivd_t[:], in_=ivd_d[:])
            dlo_t = cp.tile([P, NBLK * CB], I32)
            nc.sync.dma_start(out=dlo_t[:], in_=dlo_d[:])

            for g in range(NGRP):
                ixt = ip.tile([P, NBUCKET * (NIG // 16)], I16, tag="ix")
                nc.sync.dma_start(out=ixt[:],
                                  in_=idx_d[g * 128:(g + 1) * 128, :])
                Gts = []
                for k in range(NBUCKET):
                    Gt = gp.tile([P, BGRP * C, P], FP, tag=f"G{k}")
                    # dma_gather caps at 1024 idxs (SWDGE ring) -> split
                    # into <=8-chunk sub-gathers on 128-idx boundaries
                    for c0 in range(0, BGRP * C, 8):
                        c1 = min(c0 + 8, BGRP * C)
                        ni = (c1 - c0) * P
                        nc.gpsimd.dma_gather(
                            out_ap=Gt[:, c0:c1, :],
                            in_ap=Xg[k * BUCKET:(k + 1) * BUCKET, :],
                            idxs_ap=ixt[:, k * (NIG // 16) + c0 * 8:
                                        k * (NIG // 16) + c1 * 8],
                            num_idxs=ni, num_idxs_reg=ni, elem_size=D)
                    Gts.append(Gt)

                for bb in range(BGRP):
                    blk = g * BGRP + bb
                    nvalid = LAST_VALID if blk == NBLK - 1 else P
                    sel = sp.tile([P, CB * P], FP, tag="sel")
                    dslice = dlo_t[:, blk * CB:(blk + 1) * CB]
                    nc.vector.tensor_tensor(
                        out=sel[:].rearrange("p (c q) -> p c q", c=CB),
                        in0=dslice.unsqueeze(2).to_broadcast([P, CB, P]),
                        in1=iota_t[:].unsqueeze(1).to_broadcast([P, CB, P]),
                        op=mybir.AluOpType.is_equal)

                    xo = bp.tile([P, D], FP, tag="xo")
                    if nvalid < P:
                        nc.vector.memset(xo[:], 0.0)
                    nc.sync.dma_start(
                        out=xo[:nvalid, :],
                        in_=Xo[blk * P:blk * P + nvalid, :])
                    ptr = pp.tile([P, P], FP, space="PSUM", tag="ptr")
                    nc.tensor.transpose(out=ptr[:], in_=xo[:],
                                        identity=ident[:])
                    xoT = bp.tile([P, P], FP, tag="xoT")
                    nc.vector.tensor_copy(xoT[:], ptr[:])

                    pagg = pp.tile([P, P], FP, space="PSUM", tag="pagg")
                    for k in range(NBUCKET):
                        for j in range(C):
                            ci = k * C + j
                            nc.tensor.matmul(
                                out=pagg[:],
                                lhsT=Gts[k][:, bb * C + j, :],
                                rhs=sel[:, ci * P:(ci + 1) * P],
                                start=(ci == 0), stop=(ci == CB - 1))
                    st = bp.tile([P, P], FP, tag="st")
                    nc.vector.tensor_copy(st[:], pagg[:])

                    pown = pp.tile([P, P], FP, space="PSUM", tag="pown")
                    nc.tensor.matmul(out=pown[:], lhsT=xoT[:], rhs=Wt[:],
                                     start=True, stop=False)
                    nc.tensor.matmul(out=pown[:], lhsT=ones_t[:], rhs=br_t[:],
                                     start=False, stop=True)
                    pnbr = pp.tile([P, P], FP, space="PSUM", tag="pnbr")
                    nc.tensor.matmul(out=pnbr[:], lhsT=st[:], rhs=Wb[:],
                                     start=True, stop=True)

                    scl = bp.tile([P, P], FP, tag="scl")
                    nc.vector.tensor_scalar(
                        out=scl[:], in0=pnbr[:],
                        scalar1=ivd_t[:, blk:blk + 1], scalar2=None,
                        op0=mybir.AluOpType.mult)
                    ot = bp.tile([P, P], FP, tag="ot")
                    nc.vector.tensor_add(ot[:], scl[:], pown[:])
                    nc.vector.tensor_scalar_max(ot[:], ot[:], 0.0)
                    nc.scalar.dma_start(
                        out=out_d[blk * P:blk * P + nvalid, :],
                        in_=ot[:nvalid, :])
    nc.compile()
    return nc


_CACHE = {}


def _get_program(C):
    if C not in _CACHE:
        _CACHE[C] = _build(C)
    return _CACHE[C]


def kernel(X, src, dst, W, b):
    C, in_maps = _preprocess(X, src, dst, W, b)
    nc = _get_program(C)
    res = run_bass_kernel_spmd(nc, in_maps, core_ids=list(range(NCORES)))
    return np.concatenate([res.results[c]["out"] for c in range(NCORES)],
                          axis=0)


# revision 6
# speedup vs baseline: 2.2266x; 2.2266x over previous
"""GraphConv (scatter-mean message passing + linear + relu) on 8 trn2 cores.

Strategy (hardcoded for N=100000 nodes, D=128 feats, E=3.2M edges, 8 cores):
  - Host: sort edges by dst; shard dst nodes contiguously (12500/core).
    Per core, group edges by (128-node block, src-range bucket); 4 buckets
    of 25000 rows so row indices fit dma_gather's int16 idxs. Pad every
    (block, bucket) cell to a uniform chunk count C so one SPMD program
    serves all cores.
  - Device, per 128-node block b:
      G  = dma_gather(X[src]) rows for all chunks        [128e, C*4, 128f]
      Sel= is_equal(dst_local, iota)  one-hot             [128e, C*4*128]
      aggT (psum) = sum_j G_j^T @ Sel_j                   [128f, 128n]
      own path: X_own -> PE transpose -> X_ownT
      out = relu(X_own@W_top + b + inv_deg*(agg@W_bot))   [128n, 128fo]
  - deg is a host-side bincount folded in as inv_deg (per-partition scalar).
"""
import numpy as np

import concourse.bacc as bacc
import concourse.bass as bass
import concourse.mybir as mybir
import concourse.tile as tile
from concourse.bass_utils import run_bass_kernel_spmd
from concourse.masks import make_identity

N_NODES = 100000
D = 128
P = 128
NCORES = 8
NPC = N_NODES // NCORES          # 12500 nodes per core
NBLK = (NPC + P - 1) // P        # 98 blocks (last has 84 valid rows)
LAST_VALID = NPC - (NBLK - 1) * P  # 84
NBUCKET = 4
BUCKET = N_NODES // NBUCKET      # 25000 (< int16 max)
BGRP = 2                         # blocks per gather group
NGRP = NBLK // BGRP              # 49

FP = mybir.dt.float32
I32 = mybir.dt.int32
I16 = mybir.dt.int16


def _preprocess(X, src, dst, W, b):
    """Bucket/pad edges; build per-core device arrays. Returns (C, in_maps)."""
    src = np.ascontiguousarray(src.astype(np.int32))
    dst = np.ascontiguousarray(dst.astype(np.int32))
    X = np.ascontiguousarray(X.astype(np.float32))
    W = np.ascontiguousarray(W.astype(np.float32))
    b = np.ascontiguousarray(b.astype(np.float32)).reshape(1, D)

    deg = np.bincount(dst, minlength=N_NODES)
    inv_deg = (1.0 / np.maximum(deg, 1)).astype(np.float32)

    order = np.argsort(dst)
    src_s = src[order]
    dst_s = dst[order]
    core_bounds = np.searchsorted(dst_s, np.arange(NCORES + 1) * NPC)

    # first pass: global max chunks per (core, block, bucket) cell
    cells = []
    for c in range(NCORES):
        lo, hi = core_bounds[c], core_bounds[c + 1]
        s, d = src_s[lo:hi], dst_s[lo:hi] - c * NPC
        key = (d >> 7) * NBUCKET + np.minimum(s // BUCKET, NBUCKET - 1)
        o2 = np.argsort(key, kind="stable")
        s, d, key = s[o2], d[o2], key[o2]
        counts = np.bincount(key, minlength=NBLK * NBUCKET)
        cells.append((s, d, key, counts))
    C = max(1, int(max((cnt.max() for (_, _, _, cnt) in cells)) + 127) // 128)
    CB = C * NBUCKET            # chunks per block
    NIG = BGRP * C * P          # idxs per (group, bucket) gather

    in_maps = []
    for c in range(NCORES):
        s, d, key, counts = cells[c]
        starts = np.zeros(NBLK * NBUCKET, np.int64)
        np.cumsum(counts[:-1], out=starts[1:])
        rank = np.arange(len(s)) - np.repeat(starts, counts)
        slot = np.repeat(np.arange(NBLK * NBUCKET) * (C * P), counts) + rank

        # flat padded layout [NBLK, NBUCKET, C*P]
        idx_flat = np.zeros(NBLK * NBUCKET * C * P, np.int16)
        idx_flat[slot] = (s - (np.minimum(s // BUCKET, NBUCKET - 1) * BUCKET)
                          ).astype(np.int16)
        dlo_flat = np.full(NBLK * NBUCKET * C * P, P, np.int32)
        dlo_flat[slot] = d & 127

        # gather idxs: per (group, bucket) the list L = concat over the
        # BGRP blocks of that cell's padded idx list; position i of L maps
        # to (partition i%128, chunk i//128). SBUF layout is 16-wrapped
        # ([i%16, i//16]) replicated 8x down the 128 partitions.
        iv = idx_flat.reshape(NGRP, BGRP, NBUCKET, C * P)
        L = np.transpose(iv, (0, 2, 1, 3)).reshape(NGRP, NBUCKET, NIG)
        w16 = np.transpose(L.reshape(NGRP, NBUCKET, NIG // 16, 16),
                           (0, 1, 3, 2))              # [NGRP, NBUCKET, 16, NIG//16]
        rep = np.tile(w16, (1, 1, 8, 1))              # [NGRP, NBUCKET, 128, NIG//16]
        idx_arr = np.ascontiguousarray(
            np.transpose(rep, (0, 2, 1, 3)).reshape(NGRP * 128,
                                                    NBUCKET * (NIG // 16)))

        # dst-local one-hot ids: [128, NBLK*CB]; col b*CB + (k*C + j),
        # partition p = edge (chunk j of bucket k in block b, lane p)
        dv = dlo_flat.reshape(NBLK, CB, P)             # [b, k*C+j, p]
        dlo_arr = np.ascontiguousarray(
            np.transpose(dv, (2, 0, 1)).reshape(P, NBLK * CB))

        ivd = inv_deg[c * NPC:(c + 1) * NPC]
        ivd = np.pad(ivd, (0, NBLK * P - NPC), constant_values=1.0)
        ivd_arr = np.ascontiguousarray(ivd.reshape(NBLK, P).T)

        in_maps.append({
            "Xg": X,
            "Xo": np.ascontiguousarray(X[c * NPC:(c + 1) * NPC]),
            "idx": idx_arr,
            "dlo": dlo_arr,
            "ivd": ivd_arr,
            "Wm": W,
            "br": b,
        })
    return C, in_maps


def _build(C, repeat=1):
    CB = C * NBUCKET
    NIG = BGRP * C * P
    nc = bacc.Bacc("TRN2", target_bir_lowering=False, debug=False,
                   num_devices=NCORES)
    Xg = nc.dram_tensor("Xg", [N_NODES, D], FP, kind="ExternalInput").ap()
    Xo = nc.dram_tensor("Xo", [NPC, D], FP, kind="ExternalInput").ap()
    idx_d = nc.dram_tensor("idx", [NGRP * 128, NBUCKET * (NIG // 16)], I16,
                           kind="ExternalInput").ap()
    dlo_d = nc.dram_tensor("dlo", [P, NBLK * CB], I32,
                           kind="ExternalInput").ap()
    ivd_d = nc.dram_tensor("ivd", [P, NBLK], FP, kind="ExternalInput").ap()
    W_d = nc.dram_tensor("Wm", [2 * D, D], FP, kind="ExternalInput").ap()
    br_d = nc.dram_tensor("br", [1, D], FP, kind="ExternalInput").ap()
    out_d = nc.dram_tensor("out", [NPC, D], FP, kind="ExternalOutput").ap()

    with tile.TileContext(nc) as tc:
        with tc.tile_pool(name="const", bufs=1) as cp, \
             tc.tile_pool(name="gpool", bufs=2) as gp, \
             tc.tile_pool(name="ipool", bufs=2) as ip, \
             tc.tile_pool(name="bpool", bufs=3) as bp, \
             tc.tile_pool(name="spool", bufs=2) as sp, \
             tc.tile_pool(name="psum", bufs=2, space="PSUM") as pp:
            ident = cp.tile([P, P], FP)
            make_identity(nc, ident[:])
            iota_t = cp.tile([P, P], I32)
            nc.gpsimd.iota(iota_t[:], pattern=[[1, P]], base=0,
                           channel_multiplier=0)
            ones_t = cp.tile([1, P], FP)
            nc.vector.memset(ones_t[:], 1.0)
            Wt = cp.tile([P, D], FP)
            nc.sync.dma_start(out=Wt[:], in_=W_d[0:D, :])
            Wb = cp.tile([P, D], FP)
            nc.sync.dma_start(out=Wb[:], in_=W_d[D:2 * D, :])
            br_t = cp.tile([1, P], FP)
            nc.sync.dma_start(out=br_t[:], in_=br_d[:])
            ivd_t = cp.tile([P, NBLK], FP)
            nc.sync.dma_start(out=ivd_t[:], in_=ivd_d[:])
            dlo_t = cp.tile([P, NBLK * CB], I32)
            nc.sync.dma_start(out=dlo_t[:], in_=dlo_d[:])

            for _rep in range(repeat):
              for g in range(NGRP):
                ixt = ip.tile([P, NBUCKET * (NIG // 16)], I16, tag="ix")
                nc.sync.dma_start(out=ixt[:],
                                  in_=idx_d[g * 128:(g + 1) * 128, :])
                Gts = []
                for k in range(NBUCKET):
                    Gt = gp.tile([P, BGRP * C, P], FP, tag=f"G{k}")
                    # dma_gather caps at 1024 idxs (SWDGE ring) -> split
                    # into <=8-chunk sub-gathers on 128-idx boundaries
                    for c0 in range(0, BGRP * C, 8):
                        c1 = min(c0 + 8, BGRP * C)
                        ni = (c1 - c0) * P
                        nc.gpsimd.dma_gather(
                            out_ap=Gt[:, c0:c1, :],
                            in_ap=Xg[k * BUCKET:(k + 1) * BUCKET, :],
                            idxs_ap=ixt[:, k * (NIG // 16) + c0 * 8:
                                        k * (NIG // 16) + c1 * 8],
                            num_idxs=ni, num_idxs_reg=ni, elem_size=D)
                    Gts.append(Gt)

                for bb in range(BGRP):
                    blk = g * BGRP + bb
                    nvalid = LAST_VALID if blk == NBLK - 1 else P
                    sel = sp.tile([P, CB * P], FP, tag="sel")
                    dslice = dlo_t[:, blk * CB:(blk + 1) * CB]
                    nc.vector.tensor_tensor(
                        out=sel[:].rearrange("p (c q) -> p c q", c=CB),
                        in0=dslice.unsqueeze(2).to_broadcast([P, CB, P]),
                        in1=iota_t[:].unsqueeze(1).to_broadcast([P, CB, P]),
                        op=mybir.AluOpType.is_equal)

                    xo = bp.tile([P, D], FP, tag="xo")
                    if nvalid < P:
                        nc.vector.memset(xo[:], 0.0)
                    nc.sync.dma_start(
                        out=xo[:nvalid, :],
                        in_=Xo[blk * P:blk * P + nvalid, :])
                    ptr = pp.tile([P, P], FP, space="PSUM", tag="ptr")
                    nc.tensor.transpose(out=ptr[:], in_=xo[:],
                                        identity=ident[:])
                    xoT = bp.tile([P, P], FP, tag="xoT")
                    nc.vector.tensor_copy(xoT[:], ptr[:])

                    pagg = pp.tile([P, P], FP, space="PSUM", tag="pagg")
                    for k in range(NBUCKET):
                        for j in range(C):
                            ci = k * C + j
                            nc.tensor.matmul(
                                out=pagg[:],
                                lhsT=Gts[k][:, bb * C + j, :],
                                rhs=sel[:, ci * P:(ci + 1) * P],
                                start=(ci == 0), stop=(ci == CB - 1))
                    st = bp.tile([P, P], FP, tag="st")
                    nc.vector.tensor_copy(st[:], pagg[:])

                    pown = pp.tile([P, P], FP, space="PSUM", tag="pown")
                    nc.tensor.matmul(out=pown[:], lhsT=xoT[:], rhs=Wt[:],
                                     start=True, stop=False)
                    nc.tensor.matmul(out=pown[:], lhsT=ones_t[:], rhs=br_t[:],
                                     start=False, stop=True)
                    pnbr = pp.tile([P, P], FP, space="PSUM", tag="pnbr")
                    nc.tensor.matmul(out=pnbr[:], lhsT=st[:], rhs=Wb[:],
                                     start=True, stop=True)

                    scl = bp.tile([P, P], FP, tag="scl")
                    nc.vector.tensor_scalar(
                        out=scl[:], in0=pnbr[:],
                        scalar1=ivd_t[:, blk:blk + 1], scalar2=None,
                        op0=mybir.AluOpType.mult)
                    ot = bp.tile([P, P], FP, tag="ot")
                    nc.vector.tensor_add(ot[:], scl[:], pown[:])
                    nc.vector.tensor_scalar_max(ot[:], ot[:], 0.0)
                    nc.scalar.dma_start(
                        out=out_d[blk * P:blk * P + nvalid, :],
                        in_=ot[:nvalid, :])
    nc.compile()
    return nc


_CACHE = {}


def _get_program(C, repeat=1):
    key = (C, repeat)
    if key not in _CACHE:
        _CACHE[key] = _build(C, repeat)
    return _CACHE[key]


def kernel(X, src, dst, W, b):
    C, in_maps = _preprocess(X, src, dst, W, b)
    nc = _get_program(C)
    res = run_bass_kernel_spmd(nc, in_maps, core_ids=list(range(NCORES)))
    return np.concatenate([res.results[c]["out"] for c in range(NCORES)],
                          axis=0)


# revision 8
# speedup vs baseline: 4.5830x; 2.0583x over previous
"""GraphConv (scatter-mean message passing + linear + relu) on 8 trn2 cores.

Strategy (hardcoded for N=100000 nodes, D=128 feats, E=3.2M edges, 8 cores):
  - Host: sort edges by dst; shard dst nodes contiguously (12500/core).
    Per core, group edges by (128-node block, src-range bucket); 4 buckets
    of 25000 rows so row indices fit dma_gather's int16 idxs. Pad every
    (block, bucket) cell to a uniform chunk count C so one SPMD program
    serves all cores.
  - Device, per 128-node block b:
      G  = dma_gather(X[src]) rows for all chunks        [128e, C*4, 128f]
      Sel= is_equal(dst_local, iota)  one-hot             [128e, C*4*128]
      aggT (psum) = sum_j G_j^T @ Sel_j                   [128f, 128n]
      own path: X_own -> PE transpose -> X_ownT
      out = relu(X_own@W_top + b + inv_deg*(agg@W_bot))   [128n, 128fo]
  - deg is a host-side bincount folded in as inv_deg (per-partition scalar).
"""
import numpy as np

import concourse.bacc as bacc
import concourse.bass as bass
import concourse.mybir as mybir
import concourse.tile as tile
from concourse.bass_utils import run_bass_kernel_spmd
from concourse.masks import make_identity

N_NODES = 100000
D = 128
P = 128
NCORES = 8
NPC = N_NODES // NCORES          # 12500 nodes per core
NBLK = (NPC + P - 1) // P        # 98 blocks (last has 84 valid rows)
LAST_VALID = NPC - (NBLK - 1) * P  # 84
NBUCKET = 4
BUCKET = N_NODES // NBUCKET      # 25000 (< int16 max)
BGRP = 2                         # blocks per gather group
NGRP = NBLK // BGRP              # 49

FP = mybir.dt.float32
I32 = mybir.dt.int32
I16 = mybir.dt.int16


def _preprocess(X, src, dst, W, b):
    """Bucket/pad edges; build per-core device arrays. Returns (C, in_maps)."""
    src = np.ascontiguousarray(src.astype(np.int32))
    dst = np.ascontiguousarray(dst.astype(np.int32))
    X = np.ascontiguousarray(X.astype(np.float32))
    W = np.ascontiguousarray(W.astype(np.float32))
    b = np.ascontiguousarray(b.astype(np.float32)).reshape(1, D)

    deg = np.bincount(dst, minlength=N_NODES)
    inv_deg = (1.0 / np.maximum(deg, 1)).astype(np.float32)

    order = np.argsort(dst)
    src_s = src[order]
    dst_s = dst[order]
    core_bounds = np.searchsorted(dst_s, np.arange(NCORES + 1) * NPC)

    # first pass: global max chunks per (core, block, bucket) cell
    cells = []
    for c in range(NCORES):
        lo, hi = core_bounds[c], core_bounds[c + 1]
        s, d = src_s[lo:hi], dst_s[lo:hi] - c * NPC
        key = (d >> 7) * NBUCKET + np.minimum(s // BUCKET, NBUCKET - 1)
        o2 = np.argsort(key, kind="stable")
        s, d, key = s[o2], d[o2], key[o2]
        counts = np.bincount(key, minlength=NBLK * NBUCKET)
        cells.append((s, d, key, counts))
    C = max(1, int(max((cnt.max() for (_, _, _, cnt) in cells)) + 127) // 128)
    CB = C * NBUCKET            # chunks per block
    NIG = BGRP * C * P          # idxs per (group, bucket) gather

    in_maps = []
    for c in range(NCORES):
        s, d, key, counts = cells[c]
        starts = np.zeros(NBLK * NBUCKET, np.int64)
        np.cumsum(counts[:-1], out=starts[1:])
        rank = np.arange(len(s)) - np.repeat(starts, counts)
        slot = np.repeat(np.arange(NBLK * NBUCKET) * (C * P), counts) + rank

        # flat padded layout [NBLK, NBUCKET, C*P]
        idx_flat = np.zeros(NBLK * NBUCKET * C * P, np.int16)
        idx_flat[slot] = (s - (np.minimum(s // BUCKET, NBUCKET - 1) * BUCKET)
                          ).astype(np.int16)
        dlo_flat = np.full(NBLK * NBUCKET * C * P, P, np.int32)
        dlo_flat[slot] = d & 127

        # gather idxs: per (group, bucket) the list L = concat over the
        # BGRP blocks of that cell's padded idx list; position i of L maps
        # to (partition i%128, chunk i//128). SBUF layout is 16-wrapped
        # ([i%16, i//16]) replicated 8x down the 128 partitions.
        iv = idx_flat.reshape(NGRP, BGRP, NBUCKET, C * P)
        L = np.transpose(iv, (0, 2, 1, 3)).reshape(NGRP, NBUCKET, NIG)
        w16 = np.transpose(L.reshape(NGRP, NBUCKET, NIG // 16, 16),
                           (0, 1, 3, 2))              # [NGRP, NBUCKET, 16, NIG//16]
        rep = np.tile(w16, (1, 1, 8, 1))              # [NGRP, NBUCKET, 128, NIG//16]
        idx_arr = np.ascontiguousarray(
            np.transpose(rep, (0, 2, 1, 3)).reshape(NGRP * 128,
                                                    NBUCKET * (NIG // 16)))

        # dst-local one-hot ids: [128, NBLK*CB]; col b*CB + (k*C + j),
        # partition p = edge (chunk j of bucket k in block b, lane p)
        dv = dlo_flat.reshape(NBLK, CB, P)             # [b, k*C+j, p]
        dlo_arr = np.ascontiguousarray(
            np.transpose(dv, (2, 0, 1)).reshape(P, NBLK * CB))

        ivd = inv_deg[c * NPC:(c + 1) * NPC]
        ivd = np.pad(ivd, (0, NBLK * P - NPC), constant_values=1.0)
        ivd_arr = np.ascontiguousarray(ivd.reshape(NBLK, P).T)

        in_maps.append({
            "Xg": X,
            "Xo": np.ascontiguousarray(X[c * NPC:(c + 1) * NPC]),
            "idx": idx_arr,
            "dlo": dlo_arr,
            "ivd": ivd_arr,
            "Wm": W,
            "br": b,
        })
    return C, in_maps


def _build(C, repeat=1):
    CB = C * NBUCKET
    NIG = BGRP * C * P
    nc = bacc.Bacc("TRN2", target_bir_lowering=False, debug=False,
                   num_devices=NCORES, num_swdge_queues=4)
    Xg = nc.dram_tensor("Xg", [N_NODES, D], FP, kind="ExternalInput").ap()
    Xo = nc.dram_tensor("Xo", [NPC, D], FP, kind="ExternalInput").ap()
    idx_d = nc.dram_tensor("idx", [NGRP * 128, NBUCKET * (NIG // 16)], I16,
                           kind="ExternalInput").ap()
    dlo_d = nc.dram_tensor("dlo", [P, NBLK * CB], I32,
                           kind="ExternalInput").ap()
    ivd_d = nc.dram_tensor("ivd", [P, NBLK], FP, kind="ExternalInput").ap()
    W_d = nc.dram_tensor("Wm", [2 * D, D], FP, kind="ExternalInput").ap()
    br_d = nc.dram_tensor("br", [1, D], FP, kind="ExternalInput").ap()
    out_d = nc.dram_tensor("out", [NPC, D], FP, kind="ExternalOutput").ap()

    with tile.TileContext(nc) as tc:
        with tc.tile_pool(name="const", bufs=1) as cp, \
             tc.tile_pool(name="gpool", bufs=2) as gp, \
             tc.tile_pool(name="ipool", bufs=2) as ip, \
             tc.tile_pool(name="bpool", bufs=3) as bp, \
             tc.tile_pool(name="spool", bufs=2) as sp, \
             tc.tile_pool(name="psum", bufs=2, space="PSUM") as pp:
            ident = cp.tile([P, P], FP)
            make_identity(nc, ident[:])
            iota_t = cp.tile([P, P], I32)
            nc.gpsimd.iota(iota_t[:], pattern=[[1, P]], base=0,
                           channel_multiplier=0)
            ones_t = cp.tile([1, P], FP)
            nc.vector.memset(ones_t[:], 1.0)
            Wt = cp.tile([P, D], FP)
            nc.sync.dma_start(out=Wt[:], in_=W_d[0:D, :])
            Wb = cp.tile([P, D], FP)
            nc.sync.dma_start(out=Wb[:], in_=W_d[D:2 * D, :])
            br_t = cp.tile([1, P], FP)
            nc.sync.dma_start(out=br_t[:], in_=br_d[:])
            ivd_t = cp.tile([P, NBLK], FP)
            nc.sync.dma_start(out=ivd_t[:], in_=ivd_d[:])
            dlo_t = cp.tile([P, NBLK * CB], I32)
            nc.sync.dma_start(out=dlo_t[:], in_=dlo_d[:])

            for _rep in range(repeat):
              for g in range(NGRP):
                ixt = ip.tile([P, NBUCKET * (NIG // 16)], I16, tag="ix")
                nc.sync.dma_start(out=ixt[:],
                                  in_=idx_d[g * 128:(g + 1) * 128, :])
                Gts = []
                for k in range(NBUCKET):
                    Gt = gp.tile([P, BGRP * C, P], FP, tag=f"G{k}")
                    # dma_gather caps at 1024 idxs (SWDGE ring) -> split
                    # into <=8-chunk sub-gathers on 128-idx boundaries
                    for c0 in range(0, BGRP * C, 8):
                        c1 = min(c0 + 8, BGRP * C)
                        ni = (c1 - c0) * P
                        nc.gpsimd.dma_gather(
                            out_ap=Gt[:, c0:c1, :],
                            in_ap=Xg[k * BUCKET:(k + 1) * BUCKET, :],
                            idxs_ap=ixt[:, k * (NIG // 16) + c0 * 8:
                                        k * (NIG // 16) + c1 * 8],
                            num_idxs=ni, num_idxs_reg=ni, elem_size=D,
                            queue_num=k)
                    Gts.append(Gt)

                for bb in range(BGRP):
                    blk = g * BGRP + bb
                    nvalid = LAST_VALID if blk == NBLK - 1 else P
                    sel = sp.tile([P, CB * P], FP, tag="sel")
                    dslice = dlo_t[:, blk * CB:(blk + 1) * CB]
                    nc.vector.tensor_tensor(
                        out=sel[:].rearrange("p (c q) -> p c q", c=CB),
                        in0=dslice.unsqueeze(2).to_broadcast([P, CB, P]),
                        in1=iota_t[:].unsqueeze(1).to_broadcast([P, CB, P]),
                        op=mybir.AluOpType.is_equal)

                    xo = bp.tile([P, D], FP, tag="xo")
                    if nvalid < P:
                        nc.vector.memset(xo[:], 0.0)
                    nc.sync.dma_start(
                        out=xo[:nvalid, :],
                        in_=Xo[blk * P:blk * P + nvalid, :])
                    ptr = pp.tile([P, P], FP, space="PSUM", tag="ptr")
                    nc.tensor.transpose(out=ptr[:], in_=xo[:],
                                        identity=ident[:])
                    xoT = bp.tile([P, P], FP, tag="xoT")
                    nc.vector.tensor_copy(xoT[:], ptr[:])

                    pagg = pp.tile([P, P], FP, space="PSUM", tag="pagg")
                    for k in range(NBUCKET):
                        for j in range(C):
                            ci = k * C + j
                            nc.tensor.matmul(
                                out=pagg[:],
                                lhsT=Gts[k][:, bb * C + j, :],
                                rhs=sel[:, ci * P:(ci + 1) * P],
                                start=(ci == 0), stop=(ci == CB - 1))
                    st = bp.tile([P, P], FP, tag="st")
                    nc.vector.tensor_copy(st[:], pagg[:])

                    pown = pp.tile([P, P], FP, space="PSUM", tag="pown")
                    nc.tensor.matmul(out=pown[:], lhsT=xoT[:], rhs=Wt[:],
                                     start=True, stop=False)
                    nc.tensor.matmul(out=pown[:], lhsT=ones_t[:], rhs=br_t[:],
                                     start=False, stop=True)
                    pnbr = pp.tile([P, P], FP, space="PSUM", tag="pnbr")
                    nc.tensor.matmul(out=pnbr[:], lhsT=st[:], rhs=Wb[:],
                                     start=True, stop=True)

                    scl = bp.tile([P, P], FP, tag="scl")
                    nc.vector.tensor_scalar(
                        out=scl[:], in0=pnbr[:],
                        scalar1=ivd_t[:, blk:blk + 1], scalar2=None,
                        op0=mybir.AluOpType.mult)
                    ot = bp.tile([P, P], FP, tag="ot")
                    nc.vector.tensor_add(ot[:], scl[:], pown[:])
                    nc.vector.tensor_scalar_max(ot[:], ot[:], 0.0)
                    nc.scalar.dma_start(
                        out=out_d[blk * P:blk * P + nvalid, :],
                        in_=ot[:nvalid, :])
    nc.compile()
    return nc


_CACHE = {}


def _get_program(C, repeat=1):
    key = (C, repeat)
    if key not in _CACHE:
        _CACHE[key] = _build(C, repeat)
    return _CACHE[key]


def kernel(X, src, dst, W, b):
    C, in_maps = _preprocess(X, src, dst, W, b)
    nc = _get_program(C)
    res = run_bass_kernel_spmd(nc, in_maps, core_ids=list(range(NCORES)))
    return np.concatenate([res.results[c]["out"] for c in range(NCORES)],
                          axis=0)


# revision 10
# speedup vs baseline: 4.8958x; 1.0683x over previous
"""GraphConv (scatter-mean message passing + linear + relu) on 8 trn2 cores.

Strategy (hardcoded for N=100000 nodes, D=128 feats, E=3.2M edges, 8 cores):
  - Host: sort edges by dst; shard dst nodes contiguously (12500/core).
    Per core, group edges by (128-node block, src-range bucket); 4 buckets
    of 25000 rows so row indices fit dma_gather's int16 idxs. Pad every
    (block, bucket) cell to a uniform chunk count C so one SPMD program
    serves all cores.
  - Device, per 128-node block b:
      G  = dma_gather(X[src]) rows for all chunks        [128e, C*4, 128f]
      Sel= is_equal(dst_local, iota)  one-hot             [128e, C*4*128]
      aggT (psum) = sum_j G_j^T @ Sel_j                   [128f, 128n]
      own path: X_own -> PE transpose -> X_ownT
      out = relu(X_own@W_top + b + inv_deg*(agg@W_bot))   [128n, 128fo]
  - deg is a host-side bincount folded in as inv_deg (per-partition scalar).
"""
import numpy as np

import concourse.bacc as bacc
import concourse.bass as bass
import concourse.mybir as mybir
import concourse.tile as tile
from concourse.bass_utils import run_bass_kernel_spmd
from concourse.masks import make_identity

N_NODES = 100000
D = 128
P = 128
NCORES = 8
NPC = N_NODES // NCORES          # 12500 nodes per core
NBLK = (NPC + P - 1) // P        # 98 blocks (last has 84 valid rows)
LAST_VALID = NPC - (NBLK - 1) * P  # 84
NBUCKET = 4
BUCKET = N_NODES // NBUCKET      # 25000 (< int16 max)
BGRP = 2                         # blocks per gather group
NGRP = NBLK // BGRP              # 49

FP = mybir.dt.float32
I32 = mybir.dt.int32
I16 = mybir.dt.int16


def _preprocess(X, src, dst, W, b):
    """Bucket/pad edges; build per-core device arrays. Returns (C, in_maps)."""
    src = np.ascontiguousarray(src.astype(np.int32))
    dst = np.ascontiguousarray(dst.astype(np.int32))
    X = np.ascontiguousarray(X.astype(np.float32))
    W = np.ascontiguousarray(W.astype(np.float32))
    b = np.ascontiguousarray(b.astype(np.float32)).reshape(1, D)

    deg = np.bincount(dst, minlength=N_NODES)
    inv_deg = (1.0 / np.maximum(deg, 1)).astype(np.float32)

    order = np.argsort(dst)
    src_s = src[order]
    dst_s = dst[order]
    core_bounds = np.searchsorted(dst_s, np.arange(NCORES + 1) * NPC)

    # first pass: global max chunks per (core, block, bucket) cell
    cells = []
    for c in range(NCORES):
        lo, hi = core_bounds[c], core_bounds[c + 1]
        s, d = src_s[lo:hi], dst_s[lo:hi] - c * NPC
        key = (d >> 7) * NBUCKET + np.minimum(s // BUCKET, NBUCKET - 1)
        o2 = np.argsort(key, kind="stable")
        s, d, key = s[o2], d[o2], key[o2]
        counts = np.bincount(key, minlength=NBLK * NBUCKET)
        cells.append((s, d, key, counts))
    C = max(1, int(max((cnt.max() for (_, _, _, cnt) in cells)) + 127) // 128)
    CB = C * NBUCKET            # chunks per block
    NIG = BGRP * C * P          # idxs per (group, bucket) gather

    in_maps = []
    for c in range(NCORES):
        s, d, key, counts = cells[c]
        starts = np.zeros(NBLK * NBUCKET, np.int64)
        np.cumsum(counts[:-1], out=starts[1:])
        rank = np.arange(len(s)) - np.repeat(starts, counts)
        slot = np.repeat(np.arange(NBLK * NBUCKET) * (C * P), counts) + rank

        # flat padded layout [NBLK, NBUCKET, C*P]
        idx_flat = np.zeros(NBLK * NBUCKET * C * P, np.int16)
        idx_flat[slot] = (s - (np.minimum(s // BUCKET, NBUCKET - 1) * BUCKET)
                          ).astype(np.int16)
        dlo_flat = np.full(NBLK * NBUCKET * C * P, P, np.int32)
        dlo_flat[slot] = d & 127

        # gather idxs: per (group, bucket) the list L = concat over the
        # BGRP blocks of that cell's padded idx list; position i of L maps
        # to (partition i%128, chunk i//128). SBUF layout is 16-wrapped
        # ([i%16, i//16]) replicated 8x down the 128 partitions.
        iv = idx_flat.reshape(NGRP, BGRP, NBUCKET, C * P)
        L = np.transpose(iv, (0, 2, 1, 3)).reshape(NGRP, NBUCKET, NIG)
        w16 = np.transpose(L.reshape(NGRP, NBUCKET, NIG // 16, 16),
                           (0, 1, 3, 2))              # [NGRP, NBUCKET, 16, NIG//16]
        rep = np.tile(w16, (1, 1, 8, 1))              # [NGRP, NBUCKET, 128, NIG//16]
        idx_arr = np.ascontiguousarray(
            np.transpose(rep, (0, 2, 1, 3)).reshape(NGRP * 128,
                                                    NBUCKET * (NIG // 16)))

        # dst-local one-hot ids: [128, NBLK*CB]; col b*CB + (k*C + j),
        # partition p = edge (chunk j of bucket k in block b, lane p)
        dv = dlo_flat.reshape(NBLK, CB, P)             # [b, k*C+j, p]
        dlo_arr = np.ascontiguousarray(
            np.transpose(dv, (2, 0, 1)).reshape(P, NBLK * CB))

        ivd = inv_deg[c * NPC:(c + 1) * NPC]
        ivd = np.pad(ivd, (0, NBLK * P - NPC), constant_values=1.0)
        ivd_arr = np.ascontiguousarray(ivd.reshape(NBLK, P).T)

        in_maps.append({
            "Xg": X,
            "Xo": np.ascontiguousarray(X[c * NPC:(c + 1) * NPC]),
            "idx": idx_arr,
            "dlo": dlo_arr,
            "ivd": ivd_arr,
            "Wm": W,
            "br": b,
        })
    return C, in_maps


def _build(C, repeat=1):
    CB = C * NBUCKET
    NIG = BGRP * C * P
    nc = bacc.Bacc("TRN2", target_bir_lowering=False, debug=False,
                   num_devices=NCORES, num_swdge_queues=4)
    Xg = nc.dram_tensor("Xg", [N_NODES, D], FP, kind="ExternalInput").ap()
    Xo = nc.dram_tensor("Xo", [NPC, D], FP, kind="ExternalInput").ap()
    idx_d = nc.dram_tensor("idx", [NGRP * 128, NBUCKET * (NIG // 16)], I16,
                           kind="ExternalInput").ap()
    dlo_d = nc.dram_tensor("dlo", [P, NBLK * CB], I32,
                           kind="ExternalInput").ap()
    ivd_d = nc.dram_tensor("ivd", [P, NBLK], FP, kind="ExternalInput").ap()
    W_d = nc.dram_tensor("Wm", [2 * D, D], FP, kind="ExternalInput").ap()
    br_d = nc.dram_tensor("br", [1, D], FP, kind="ExternalInput").ap()
    out_d = nc.dram_tensor("out", [NPC, D], FP, kind="ExternalOutput").ap()

    with tile.TileContext(nc) as tc:
        with tc.tile_pool(name="const", bufs=1) as cp, \
             tc.tile_pool(name="gpool", bufs=2) as gp, \
             tc.tile_pool(name="ipool", bufs=2) as ip, \
             tc.tile_pool(name="bpool", bufs=3) as bp, \
             tc.tile_pool(name="spool", bufs=2) as sp, \
             tc.tile_pool(name="psum", bufs=2, space="PSUM") as pp:
            ident = cp.tile([P, P], FP)
            make_identity(nc, ident[:])
            iota_t = cp.tile([P, P], I32)
            nc.gpsimd.iota(iota_t[:], pattern=[[1, P]], base=0,
                           channel_multiplier=0)
            ones_t = cp.tile([1, P], FP)
            nc.vector.memset(ones_t[:], 1.0)
            Wt = cp.tile([P, D], FP)
            nc.sync.dma_start(out=Wt[:], in_=W_d[0:D, :])
            Wb = cp.tile([P, D], FP)
            nc.sync.dma_start(out=Wb[:], in_=W_d[D:2 * D, :])
            br_t = cp.tile([1, P], FP)
            nc.sync.dma_start(out=br_t[:], in_=br_d[:])
            ivd_t = cp.tile([P, NBLK], FP)
            nc.sync.dma_start(out=ivd_t[:], in_=ivd_d[:])
            dlo_t = cp.tile([P, NBLK * CB], I32)
            nc.sync.dma_start(out=dlo_t[:], in_=dlo_d[:])

            for _rep in range(repeat):
              for g in range(NGRP):
                ixt = ip.tile([P, NBUCKET * (NIG // 16)], I16, tag="ix")
                nc.sync.dma_start(out=ixt[:],
                                  in_=idx_d[g * 128:(g + 1) * 128, :])
                Gts = []
                for k in range(NBUCKET):
                    Gt = gp.tile([P, BGRP * C, P], FP, tag=f"G{k}")
                    # dma_gather caps at 1024 idxs (SWDGE ring) -> split
                    # into <=8-chunk sub-gathers on 128-idx boundaries
                    for c0 in range(0, BGRP * C, 8):
                        c1 = min(c0 + 8, BGRP * C)
                        ni = (c1 - c0) * P
                        nc.gpsimd.dma_gather(
                            out_ap=Gt[:, c0:c1, :],
                            in_ap=Xg[k * BUCKET:(k + 1) * BUCKET, :],
                            idxs_ap=ixt[:, k * (NIG // 16) + c0 * 8:
                                        k * (NIG // 16) + c1 * 8],
                            num_idxs=ni, num_idxs_reg=ni, elem_size=D,
                            queue_num=k)
                    Gts.append(Gt)

                for bb in range(BGRP):
                    blk = g * BGRP + bb
                    nvalid = LAST_VALID if blk == NBLK - 1 else P
                    sel = sp.tile([P, CB * P], FP, tag="sel")
                    dslice = dlo_t[:, blk * CB:(blk + 1) * CB]
                    nc.vector.tensor_tensor(
                        out=sel[:].rearrange("p (c q) -> p c q", c=CB),
                        in0=dslice.unsqueeze(2).to_broadcast([P, CB, P]),
                        in1=iota_t[:].unsqueeze(1).to_broadcast([P, CB, P]),
                        op=mybir.AluOpType.is_equal)

                    xo = bp.tile([P, D], FP, tag="xo")
                    if nvalid < P:
                        nc.vector.memset(xo[:], 0.0)
                    nc.sync.dma_start(
                        out=xo[:nvalid, :],
                        in_=Xo[blk * P:blk * P + nvalid, :])
                    ptr = pp.tile([P, P], FP, space="PSUM", tag="ptr")
                    nc.tensor.transpose(out=ptr[:], in_=xo[:],
                                        identity=ident[:])
                    xoT = bp.tile([P, P], FP, tag="xoT")
                    nc.vector.tensor_copy(xoT[:], ptr[:])

                    pagg = pp.tile([P, P], FP, space="PSUM", tag="pagg")
                    for k in range(NBUCKET):
                        for j in range(C):
                            ci = k * C + j
                            nc.tensor.matmul(
                                out=pagg[:],
                                lhsT=Gts[k][:, bb * C + j, :],
                                rhs=sel[:, ci * P:(ci + 1) * P],
                                start=(ci == 0), stop=(ci == CB - 1))
                    st = bp.tile([P, P], FP, tag="st")
                    nc.vector.tensor_copy(st[:], pagg[:])

                    pown = pp.tile([P, P], FP, space="PSUM", tag="pown")
                    nc.tensor.matmul(out=pown[:], lhsT=xoT[:], rhs=Wt[:],
                                     start=True, stop=False)
                    nc.tensor.matmul(out=pown[:], lhsT=ones_t[:], rhs=br_t[:],
                                     start=False, stop=True)
                    pnbr = pp.tile([P, P], FP, space="PSUM", tag="pnbr")
                    nc.tensor.matmul(out=pnbr[:], lhsT=st[:], rhs=Wb[:],
                                     start=True, stop=True)

                    scl = bp.tile([P, P], FP, tag="scl")
                    nc.vector.tensor_scalar(
                        out=scl[:], in0=pnbr[:],
                        scalar1=ivd_t[:, blk:blk + 1], scalar2=None,
                        op0=mybir.AluOpType.mult)
                    ot = bp.tile([P, P], FP, tag="ot")
                    nc.vector.tensor_add(ot[:], scl[:], pown[:])
                    nc.vector.tensor_scalar_max(ot[:], ot[:], 0.0)
                    nc.scalar.dma_start(
                        out=out_d[blk * P:blk * P + nvalid, :],
                        in_=ot[:nvalid, :])
    nc.compile()
    return nc


_CACHE = {}


def _get_program(C, repeat=1):
    key = (C, repeat)
    if key not in _CACHE:
        _CACHE[key] = _build(C, repeat)
    return _CACHE[key]


def kernel(X, src, dst, W, b):
    C, in_maps = _preprocess(X, src, dst, W, b)
    nc = _get_program(C)
    res = run_bass_kernel_spmd(nc, in_maps, core_ids=list(range(NCORES)))
    return np.concatenate([res.results[c]["out"] for c in range(NCORES)],
                          axis=0)
